# revision 10
# baseline (speedup 1.0000x reference)
"""DiM block (Mamba-style selective-scan transformer block) on 8 TRN2 cores.

Sharding: core i handles (b = i//4, k = i%4) — one batch sample and one of
the 4 scan directions. The spatial permutation q_k is pushed onto
host-prepared inputs so ONE SPMD program serves all 8 cores.

vs the original version:
- Depthwise 3x3 conv is folded into the input projection: host precomputes
  per-tap weights W9[tap] = W_in_xi * conv_w[tap], the kernel matmuls 9
  shifted slices of a zero-padded (guarded) hT tile, accumulating in PSUM.
  The conv bias validity at image borders rides a 9-partition indicator
  matmul (b9 x ind9).
- Scan runs j-major (d-group outer) with y accumulated in PSUM across all
  chunks; each j's ys is AllGathered as soon as it finishes so 3 of the 4
  collectives overlap the remaining scan.
- LN stats are computed with PE ones-matmuls (partition reduction) instead
  of loading extra x copies; the z/silu branch and the post-phase weight
  loads are deferred into the scan window.
- Weight loads use gpsimd casting DMAs (f32 dram -> bf16 SBUF directly).
"""
import json
import sys

sys.path.insert(0, "/opt/trn_rl_repo")

import numpy as np
import concourse.bass as bass
import concourse.mybir as mybir
import concourse.tile as tile
from concourse.bass_utils import run_bass_kernel_spmd

# ---------------------------------------------------------------------------
# Workaround: this walrus build rejects instructions carrying >1 embedded
# sem-wait. Split extra waits onto same-engine NoOps at BIR serialization.
_MAXW = 1
_wsplit_counter = [0]


def _split_multi_waits(bir: dict) -> dict:
    for fn in bir.get("functions", []):
        for bb in fn.get("blocks", []):
            insts = bb.get("instructions", [])
            if not any(
                len((i.get("sync_info") or {}).get("on_wait") or []) > _MAXW
                for i in insts
            ):
                continue
            out = []
            for inst in insts:
                si = inst.get("sync_info")
                waits = (si or {}).get("on_wait") or []
                if len(waits) > _MAXW and inst.get("engine"):
                    for w in waits[:-_MAXW]:
                        _wsplit_counter[0] += 1
                        out.append({
                            "debug": inst.get("debug", 0),
                            "engine": inst["engine"],
                            "ins": [], "outs": [],
                            "name": f"I-wsplit-{_wsplit_counter[0]}",
                            "opcode": "NoOp",
                            "sync_info": {"on_update": [], "on_wait": [w]},
                        })
                    si["on_wait"] = waits[-_MAXW:]
                out.append(inst)
            bb["instructions"] = out
    return bir


_orig_to_json_bytes = bass.Bass.to_json_bytes


def _patched_to_json_bytes(self) -> bytes:
    j = json.loads(_orig_to_json_bytes(self))
    _split_multi_waits(j)
    return json.dumps(j).encode()


bass.Bass.to_json_bytes = _patched_to_json_bytes

# ---------------------------------------------------------------------------
B, Hs, Ws, DIM = 2, 32, 32, 256
L = Hs * Ws
DI = 2 * DIM
DS = 64
DTR = DIM // 16
K = 4
HID = 4 * DIM

f32 = mybir.dt.float32
bf16 = mybir.dt.bfloat16
MUL = mybir.AluOpType.mult
ADD = mybir.AluOpType.add
SUB = mybir.AluOpType.subtract
BYP = mybir.AluOpType.bypass
AF = mybir.ActivationFunctionType
AX = mybir.AxisListType

EPS = 1e-6
NPAIRS = DS // 2          # 32 n-pairs
CHUNK = 4                 # n-pairs per chunk
NCHUNK = NPAIRS // CHUNK  # 8

G = 35                    # guard columns around the padded conv image
PADW = 34 * 34            # 1156
# smallpack column map
SP_BZ, SP_CVB, SP_DTB, SP_DP, SP_LNW, SP_LNB = 0, 4, 8, 12, 16, 20
SP_BOUT, SP_BFC1, SP_BFC2, SP_ALOG = 24, 26, 34, 36
SP_COLS = 36 + 256

XIN_DVE_MOD = 7           # 1-in-7 xin muls run on DVE, rest on Pool


def build_program():
    nc = bass.Bass()

    def din(name, shape, dt=f32):
        return nc.dram_tensor(name, list(shape), dt, kind="ExternalInput")

    T = {}
    T["xT_pre"] = din("xT_pre", (DIM, L))
    T["xT_row"] = din("xT_row", (DIM, L))
    T["c_vec"] = din("c_vec", (1, DIM))
    T["W_ada"] = din("W_ada", (DIM, 6 * DIM))
    T["b_ada"] = din("b_ada", (1, 6 * DIM))
    T["W9"] = din("W9", (9, DIM, DI))         # conv-tap-folded W_in_xi
    T["b9"] = din("b9", (9, DI))              # conv-tap-folded b_in_xi
    T["ind9"] = din("ind9", (9, PADW))        # shifted interior indicators
    T["W_in_z"] = din("W_in_z", (DIM, DI))
    T["W_xp"] = din("W_xp", (DI, 144))        # cols reordered [B(64), C(64), dtr(16)]
    T["W_dtm"] = din("W_dtm", (DTR, DI))
    T["smallpack"] = din("smallpack", (128, SP_COLS))
    T["sel2"] = din("sel2", (2, 128, 128))    # [par][k,p] = (k == p%64 + par*64)
    T["ysel"] = din("ysel", (128, 64))        # [p, d] = (p%64 == d)
    T["ident"] = din("ident", (128, 128))
    T["W_out"] = din("W_out", (DI, DIM))
    T["W_fc1"] = din("W_fc1", (DIM, HID))
    T["W_fc2"] = din("W_fc2", (HID, DIM))

    T["outT"] = nc.dram_tensor("outT", [DIM, L], f32, kind="ExternalOutput")
    for j in range(4):
        T[f"ys_l{j}"] = nc.dram_tensor(f"ys_l{j}", [128, L], bf16)
        T[f"ys_g{j}"] = nc.dram_tensor(f"ys_g{j}", [4, 128, L], bf16)
    T["mod_scr"] = nc.dram_tensor("mod_scr", [1792], f32)

    with tile.TileContext(nc) as tc:
        _build_body(nc, tc, T)
    return nc


def _build_body(nc, tc, T):
    from contextlib import ExitStack

    dma = nc.sync.dma_start
    gdma = nc.gpsimd.dma_start

    perstack = ExitStack()
    persist = perstack.enter_context(tc.tile_pool(name="persist", bufs=1))
    wstack = ExitStack()
    wp = wstack.enter_context(tc.tile_pool(name="weights", bufs=1))
    prestack = ExitStack()
    prew = prestack.enter_context(tc.tile_pool(name="prew", bufs=1))
    work = prestack.enter_context(tc.tile_pool(name="work", bufs=1))
    pre_ps = ExitStack()
    psA = pre_ps.enter_context(tc.tile_pool(name="ps_pre", bufs=1, space="PSUM"))

    # ---------------- S0: loads ------------------------------------------
    c_t = work.tile([1, DIM], f32, tag="c_t", name="c_t")
    dma(c_t[:], T["c_vec"][:, :])
    smallt = persist.tile([128, SP_COLS], f32, tag="smallt", name="smallt")
    dma(smallt[:], T["smallpack"][:, :])
    Wada = []
    for j in range(2):
        tb = prew.tile([128, 6 * DIM], bf16, tag=f"Wada_b{j}", name=f"Wada_b{j}")
        gdma(tb[:], T["W_ada"][j * 128:(j + 1) * 128, :])
        Wada.append(tb)
    bada = work.tile([1, 6 * DIM], f32, tag="bada", name="bada")
    dma(bada[:], T["b_ada"][:, :])
    xTp = []
    for cc in range(2):
        xt = prew.tile([128, L], bf16, tag=f"xTp{cc}", name=f"xTp{cc}")
        gdma(xt[:], T["xT_pre"][cc * 128:(cc + 1) * 128, :])
        xTp.append(xt)
    identf = persist.tile([128, 128], f32, tag="identf", name="identf")
    dma(identf[:], T["ident"][:, :])
    W9b = []
    for tap in range(9):
        for kk in range(2):
            tb = prew.tile([128, DI], bf16, tag=f"W9_{tap}_{kk}", name=f"W9_{tap}_{kk}")
            gdma(tb[:], T["W9"][tap, kk * 128:(kk + 1) * 128, :])
            W9b.append(tb)
    b9b = prew.tile([9, DI], bf16, tag="b9b", name="b9b")
    gdma(b9b[:], T["b9"][:, :])
    ind9b = prew.tile([9, PADW], bf16, tag="ind9b", name="ind9b")
    gdma(ind9b[:], T["ind9"][:, :])
    sel_b = []
    for par in range(2):
        tb = persist.tile([128, 128], bf16, tag=f"sel_b{par}", name=f"sel_b{par}")
        gdma(tb[:], T["sel2"][par, :, :])
        sel_b.append(tb)
    ysel_b = persist.tile([128, 64], bf16, tag="ysel_b", name="ysel_b")
    gdma(ysel_b[:], T["ysel"][:, :])
    Wxp = []
    for kk in range(4):
        tb = persist.tile([128, 144], bf16, tag=f"Wxp_b{kk}", name=f"Wxp_b{kk}")
        gdma(tb[:], T["W_xp"][kk * 128:(kk + 1) * 128, :])
        Wxp.append(tb)
    Wdt = persist.tile([DTR, DI], bf16, tag="Wdt_b", name="Wdt_b")
    gdma(Wdt[:], T["W_dtm"][:, :])
    Wz = []
    for kk in range(2):
        tb = persist.tile([128, DI], bf16, tag=f"Wz_b{kk}", name=f"Wz_b{kk}")
        gdma(tb[:], T["W_in_z"][kk * 128:(kk + 1) * 128, :])
        Wz.append(tb)
    xTrb = []
    for cc in range(2):
        xt = persist.tile([128, L], bf16, tag=f"xTrb{cc}", name=f"xTrb{cc}")
        gdma(xt[:], T["xT_row"][cc * 128:(cc + 1) * 128, :])
        xTrb.append(xt)

    eps_col = persist.tile([128, 1], f32, tag="eps_col", name="eps_col")
    nc.gpsimd.memset(eps_col[:], EPS)
    ones_b = persist.tile([128, 1], bf16, tag="ones_b", name="ones_b")
    nc.gpsimd.memset(ones_b[:], 1.0)
    ones_f = persist.tile([128, 1], f32, tag="ones_f", name="ones_f")
    nc.gpsimd.memset(ones_f[:], 1.0)
    acols = persist.tile([128, 256], f32, tag="acols", name="acols")
    nc.scalar.activation(acols[:], smallt[:, SP_ALOG:SP_ALOG + 256], AF.Exp)

    # ---------------- S1: adaLN modulation vector -------------------------
    c_silu = work.tile([1, DIM], f32, tag="c_silu", name="c_silu")
    nc.scalar.activation(c_silu[:], c_t[:], AF.Silu)
    c_col = work.tile([128, 2], f32, tag="c_col", name="c_col")
    dma(T["mod_scr"][1536:1792], c_silu[0:1, :])
    dma(c_col[:], T["mod_scr"][1536:1792].rearrange("(j p) -> p j", j=2, p=128))
    c_colb = work.tile([128, 2], bf16, tag="c_colb", name="c_colb")
    nc.vector.tensor_copy(c_colb[:], c_col[:])

    mod = work.tile([1, 6 * DIM], f32, tag="mod", name="mod")
    for fb in range(3):
        pmod = psA.tile([1, 512], f32, tag="pmod", name=f"pmod{fb}", bufs=1)
        for kk in range(2):
            nc.tensor.matmul(pmod[:], c_colb[:, kk:kk + 1],
                             Wada[kk][:, fb * 512:(fb + 1) * 512],
                             start=(kk == 0), stop=(kk == 1))
        nc.vector.tensor_tensor(mod[:, fb * 512:(fb + 1) * 512], pmod[:],
                                bada[:, fb * 512:(fb + 1) * 512], ADD)
    dma(T["mod_scr"][0:1536], mod[0:1, :])
    mcolt = persist.tile([128, 12], f32, tag="mcolt", name="mcolt")
    dma(mcolt[:], T["mod_scr"][0:1536].rearrange("(a p) -> p a", a=12, p=128))

    def mcol(i6, cc):
        return mcolt[:, i6 * 2 + cc:i6 * 2 + cc + 1]

    s1_msa = persist.tile([128, 2], f32, tag="s1_msa", name="s1_msa")
    nc.scalar.activation(s1_msa[:], mcolt[:, 2:4], AF.Identity, bias=1.0)
    s1_mlp = persist.tile([128, 2], f32, tag="s1_mlp", name="s1_mlp")
    nc.scalar.activation(s1_mlp[:], mcolt[:, 8:10], AF.Identity, bias=1.0)
    gb_out = persist.tile([128, 2], f32, tag="gb_out", name="gb_out")
    nc.vector.tensor_tensor(gb_out[:], mcolt[:, 4:6],
                            smallt[:, SP_BOUT:SP_BOUT + 2], MUL)
    gb_fc2 = persist.tile([128, 2], f32, tag="gb_fc2", name="gb_fc2")
    nc.vector.tensor_tensor(gb_fc2[:], mcolt[:, 10:12],
                            smallt[:, SP_BFC2:SP_BFC2 + 2], MUL)

    # ---------------- shared LN helpers (stats via PE) --------------------
    def pe_stats(tiles, onecol, ssum, ssq, pool, name, sq_dt, accum=None):
        """Per-token sum and sum-of-squares over partitions of `tiles`.
        tiles: list of (128, L) tiles whose partitions are feature rows.
        Writes into PSUM tiles ssum/ssq (128, 8). accum: (start, stop) flags
        override for cross-call accumulation."""
        n = len(tiles)
        sqt = []
        for i, t in enumerate(tiles):
            sq = pool.tile([128, L], sq_dt, tag=f"sqt{sq_dt}", name=f"sq_{name}{i}",
                           bufs=2)
            nc.scalar.activation(sq[:], t[:], AF.Square)
            sqt.append(sq)
        st0, st1 = (True, True) if accum is None else accum
        for tb in range(8):
            for i in range(n):
                nc.tensor.matmul(ssum[:, tb:tb + 1],
                                 tiles[i][:, tb * 128:(tb + 1) * 128], onecol[:],
                                 start=(st0 and i == 0), stop=(st1 and i == n - 1))
                nc.tensor.matmul(ssq[:, tb:tb + 1],
                                 sqt[i][:, tb * 128:(tb + 1) * 128], onecol[:],
                                 start=(st0 and i == 0), stop=(st1 and i == n - 1))

    def bcast_cols(stat, name, pool, psum_pool, tag, dt):
        """(128,8) per-token stat -> (128,L) all-partition broadcast tile."""
        statT_p = psum_pool.tile([8, 128], f32, tag="statT_p", name=f"sTp_{name}",
                                 bufs=1)
        nc.tensor.transpose(statT_p[:], stat[:], identf[:])
        statT = pool.tile([8, 128], dt, tag=f"statT{dt}", name=f"sT_{name}", bufs=1)
        nc.scalar.copy(statT[:], statT_p[:])
        row2 = pool.tile([2, L], dt, tag=f"row2{dt}", name=f"r2_{name}", bufs=1)
        dma(row2[0:1, :], statT[:, :])
        dma(row2[1:2, :], statT[:, :])
        bc = pool.tile([128, L], dt, tag=f"{tag}{dt}", name=f"bc_{name}", bufs=1)
        dma(bc[:], row2[:, :].partition_broadcast(64).rearrange("n d f -> d n f"))
        return bc

    def stats_tail(ssum, ssq, dim, name, pool, psum_pool, dt):
        mu = pool.tile([128, 8], f32, tag="pmu", name=f"pmu_{name}", bufs=2)
        nc.vector.tensor_scalar_mul(mu[:], ssum[:], 1.0 / dim)
        mu2 = pool.tile([128, 8], f32, tag="pmu2", name=f"pmu2_{name}", bufs=2)
        nc.vector.tensor_tensor(mu2[:], mu[:], mu[:], MUL)
        var = pool.tile([128, 8], f32, tag="pvar", name=f"pvar_{name}", bufs=2)
        nc.vector.scalar_tensor_tensor(var[:], ssq[:], 1.0 / dim, mu2[:], MUL, SUB)
        std = pool.tile([128, 8], f32, tag="pstd", name=f"pstd_{name}", bufs=2)
        nc.scalar.activation(std[:], var[:], AF.Sqrt, bias=eps_col[:, 0:1])
        rstd = pool.tile([128, 8], f32, tag="prstd", name=f"prstd_{name}", bufs=2)
        nc.vector.reciprocal(rstd[:], std[:])
        mu_bc = bcast_cols(mu, f"{name}m", pool, psum_pool, "bcA", dt)
        rstd_bc = bcast_cols(rstd, f"{name}r", pool, psum_pool, "bcB", dt)
        return mu_bc, rstd_bc

    # ---------------- S2: LN1(pre) + modulate into padded tiles ----------
    ssum_p = psA.tile([128, 8], f32, tag="ssum_p", name="ssum_p")
    ssq_p = psA.tile([128, 8], f32, tag="ssq_p", name="ssq_p")
    pe_stats(xTp, ones_b, ssum_p, ssq_p, work, "p", bf16)
    mu_p, rstd_p = stats_tail(ssum_p, ssq_p, DIM, "p", work, psA, bf16)

    hpad = []
    for cc in range(2):
        hp = prew.tile([128, G + PADW + G], bf16, tag=f"hpad{cc}", name=f"hpad{cc}")
        nc.gpsimd.memset(hp[:], 0.0)
        t1 = work.tile([128, L], bf16, tag="hscr", name=f"hs1_p{cc}", bufs=2)
        nc.vector.tensor_tensor(t1[:], xTp[cc][:], mu_p[:], SUB)
        t2 = work.tile([128, L], bf16, tag="hscr", name=f"hs2_p{cc}", bufs=2)
        nc.vector.tensor_tensor(t2[:], t1[:], rstd_p[:], MUL)
        interior = (hp[:, G:G + PADW]
                    .rearrange("p (H W) -> p H W", H=34, W=34)[:, 1:33, 1:33])
        nc.scalar.activation(interior, t2[:].rearrange("p (h w) -> p h w", h=32, w=32),
                             AF.Identity, bias=mcol(0, cc), scale=s1_msa[:, cc:cc + 1])
        hpad.append(hp)

    pre_ps.close()

    # ---------------- S4: conv-proj on PE + B/C/dtr projections -----------
    conv_ps = ExitStack()
    cpp = conv_ps.enter_context(tc.tile_pool(name="ps_conv", bufs=1, space="PSUM"))
    bcpp = conv_ps.enter_context(tc.tile_pool(name="ps_bc", bufs=1, space="PSUM"))
    CCHUNKS = [(0, 512), (512, 1024), (1024, PADW)]
    OFFS = [(dy - 1) * 34 + (dx - 1) for dy in range(3) for dx in range(3)]

    ppbc = bcpp.tile([128, L], f32, tag="ppbc", name="ppbc")
    ppdtr = bcpp.tile([DTR, L], f32, tag="ppdtr", name="ppdtr")
    xsT = []
    for j in range(4):
        ppcv = cpp.tile([128, PADW], f32, tag="ppcv", name=f"ppcv{j}", bufs=1)
        for (c0, c1) in CCHUNKS:
            nmm = 0
            for tap in range(9):
                for kk in range(2):
                    nc.tensor.matmul(
                        ppcv[:, c0:c1],
                        W9b[tap * 2 + kk][:, j * 128:(j + 1) * 128],
                        hpad[kk][:, G + c0 + OFFS[tap]:G + c1 + OFFS[tap]],
                        start=(nmm == 0), stop=False)
                    nmm += 1
            nc.tensor.matmul(ppcv[:, c0:c1], b9b[:, j * 128:(j + 1) * 128],
                             ind9b[:, c0:c1], start=False, stop=True)
        xs = wp.tile([128, L], bf16, tag=f"xsT{j}", name=f"xsT{j}")
        inter = (ppcv[:, :].rearrange("p (H W) -> p H W", H=34, W=34)
                 [:, 1:33, 1:33])
        nc.scalar.activation(xs[:].rearrange("p (h w) -> p h w", h=32, w=32),
                             inter, AF.Silu,
                             bias=smallt[:, SP_CVB + j:SP_CVB + j + 1])
        xsT.append(xs)
        # B/C and dt_r projections accumulate over j
        for th in range(2):
            nc.tensor.matmul(ppbc[:, th * 512:(th + 1) * 512],
                             Wxp[j][:, 0:128],
                             xs[:, th * 512:(th + 1) * 512],
                             start=(j == 0), stop=(j == 3))
            nc.tensor.matmul(ppdtr[:, th * 512:(th + 1) * 512],
                             Wxp[j][:, 128:144],
                             xs[:, th * 512:(th + 1) * 512],
                             start=(j == 0), stop=(j == 3))

    bc_t = wp.tile([128, L], bf16, tag="bc_t", name="bc_t")
    nc.scalar.copy(bc_t[:], ppbc[:])
    dtr_t = wp.tile([DTR, L], bf16, tag="dtr_t", name="dtr_t")
    nc.scalar.copy(dtr_t[:], ppdtr[:])
    conv_ps.close()
    prestack.close()

    # ---------------- S7/S8: scan stage, j-major --------------------------
    scan_st = ExitStack()
    argp = scan_st.enter_context(tc.tile_pool(name="argp", bufs=2, space="PSUM"))
    yps = scan_st.enter_context(tc.tile_pool(name="yps", bufs=1, space="PSUM"))
    stp = scan_st.enter_context(tc.tile_pool(name="stp", bufs=1, space="PSUM"))
    spool = scan_st.enter_context(tc.tile_pool(name="spool", bufs=2))
    bpool = scan_st.enter_context(tc.tile_pool(name="bpool", bufs=1))
    ypool = scan_st.enter_context(tc.tile_pool(name="ypool", bufs=1))

    statsp = stp.tile([128, 48], f32, tag="statsp", name="statsp")
    ly_ssum = statsp[:, 0:8]
    ly_ssq = statsp[:, 8:16]

    y_row = [None] * 4
    siluz = [None] * 4

    it = 0
    for j in range(4):
        # --- dt chain for this j
        ppd = argp.tile([128, L], f32, tag="arg", name=f"ppdt{j}")
        for th in range(2):
            nc.tensor.matmul(ppd[:, th * 512:(th + 1) * 512],
                             Wdt[:, j * 128:(j + 1) * 128],
                             dtr_t[:, th * 512:(th + 1) * 512],
                             start=True, stop=True)
        spx = spool.tile([128, L], f32, tag="spx", name=f"spx{j}", bufs=1)
        nc.scalar.activation(spx[:], ppd[:], AF.Exp,
                             bias=smallt[:, SP_DTB + j:SP_DTB + j + 1])
        dt_b = spool.tile([128, L], bf16, tag="dtT", name=f"dtT{j}", bufs=2)
        nc.scalar.activation(dt_b[:], spx[:], AF.Ln, bias=1.0)
        ndt = ypool.tile([128, L], bf16, tag="ndt", name=f"ndtT{j}", bufs=2)
        nc.vector.tensor_scalar_mul(ndt[:], dt_b[:], -1.0)
        w_b = spool.tile([128, L], bf16, tag="wTtmp", name=f"wT{j}", bufs=2)
        nc.vector.tensor_tensor(w_b[:], dt_b[:], xsT[j][:], MUL)
        wbc = {}
        for par in range(2):
            g = 2 * j + par
            wsrc = w_b[par * 64:par * 64 + 64, :]
            wb = ypool.tile([128, L], bf16, tag=f"wbc{par}", name=f"wbc{g}", bufs=1)
            dma(wb[0:64, :], wsrc)
            dma(wb[64:128, :], wsrc)
            wbc[par] = wb

        ypt = yps.tile([128, L], f32, tag="ypt", name=f"ypt{j}")
        for ch in range(NCHUNK):
            Bb, Cb = [], []
            for il in range(CHUNK):
                i = ch * CHUNK + il
                bbt = bpool.tile([128, L], bf16, tag="Bb", name=f"Bb{j}_{ch}_{il}",
                                 bufs=6)
                dma(bbt[:], bc_t[2 * i:2 * i + 2, :]
                    .partition_broadcast(64).rearrange("d n f -> n d f"))
                Bb.append(bbt)
                cbt = bpool.tile([128, L], bf16, tag="Cb", name=f"Cb{j}_{ch}_{il}",
                                 bufs=6)
                dma(cbt[:], bc_t[64 + 2 * i:64 + 2 * i + 2, :]
                    .partition_broadcast(64).rearrange("d n f -> n d f"))
                Cb.append(cbt)
            for par in range(2):
                g = 2 * j + par
                for il in range(CHUNK):
                    i = ch * CHUNK + il
                    arg = argp.tile([128, L], f32, tag="arg", name=f"arg{it}")
                    for th in range(2):
                        nc.tensor.matmul(arg[:, th * 512:(th + 1) * 512],
                                         sel_b[par][:],
                                         ndt[:, th * 512:(th + 1) * 512],
                                         start=True, stop=True)
                    dA = spool.tile([128, L], bf16, tag="dA", name=f"dA{it}", bufs=3)
                    nc.scalar.activation(dA[:], arg[:], AF.Exp,
                                         scale=acols[:, g * 32 + i:g * 32 + i + 1])
                    xin = spool.tile([128, L], bf16, tag="xin", name=f"xin{it}",
                                     bufs=4)
                    xeng = nc.vector if (it % XIN_DVE_MOD == 3) else nc.gpsimd
                    xeng.tensor_tensor(xin[:], wbc[par][:], Bb[il][:], MUL)
                    h = spool.tile([128, L], bf16, tag="h", name=f"h{it}", bufs=3)
                    nc.vector.tensor_tensor_scan(h[:], dA[:], xin[:], 0.0, MUL, ADD)
                    yc = spool.tile([128, L], bf16, tag="yc", name=f"yc{it}", bufs=3)
                    nc.vector.tensor_tensor(yc[:], h[:], Cb[il][:], MUL)
                    for th in range(2):
                        nc.tensor.matmul(
                            ypt[par * 64:par * 64 + 64, th * 512:(th + 1) * 512],
                            ysel_b[:], yc[:, th * 512:(th + 1) * 512],
                            start=(ch == 0 and il == 0),
                            stop=(ch == NCHUNK - 1 and il == CHUNK - 1))
                    it += 1

        # --- ys for this j -> DRAM -> AllGather
        ys16 = spool.tile([128, L], bf16, tag="ys16", name=f"ys16_{j}", bufs=2)
        nc.vector.scalar_tensor_tensor(ys16[:], xsT[j][:],
                                       smallt[:, SP_DP + j:SP_DP + j + 1],
                                       ypt[:], MUL, ADD)
        dma(T[f"ys_l{j}"][:, :], ys16[:])
        nc.gpsimd.collective_compute(
            "AllGather", BYP,
            replica_groups=[[0, 1, 2, 3], [4, 5, 6, 7]],
            ins=[T[f"ys_l{j}"][:, :]],
            outs=[T[f"ys_g{j}"][:, :, :]],
        )

        # --- combine the 4 directions for this j (overlaps later j scans)
        ysk = []
        for k in range(4):
            t16 = ypool.tile([128, L], bf16, tag="ysk", name=f"ysk{j}_{k}", bufs=4)
            dma(t16[:], T[f"ys_g{j}"][k, :, :])
            ysk.append(t16)
        acc = ypool.tile([128, L], bf16, tag="yrow_s", name=f"yrow{j}_0", bufs=2)
        nc.vector.tensor_copy(acc[:], ysk[0][:])
        for k in (1, 2, 3):
            nacc = ypool.tile([128, L], bf16,
                              tag=f"yrowf{j}" if k == 3 else "yrow_s",
                              name=f"yrow{j}_{k}", bufs=1 if k == 3 else 2)
            ceng = nc.vector
            if k == 2:
                ceng.tensor_tensor(nacc[:], acc[:], ysk[2][:, ::-1], ADD)
            else:
                srct = ysk[k][:, :] if k == 1 else ysk[k][:, ::-1]
                view = (srct.rearrange("p (w h) -> p w h", w=32, h=32)
                        .rearrange("p w h -> p h w"))
                ceng.tensor_tensor(
                    nacc[:].rearrange("p (h w) -> p h w", h=32, w=32),
                    acc[:].rearrange("p (h w) -> p h w", h=32, w=32),
                    view, ADD)
            acc = nacc
        y_row[j] = acc
        pe_stats([acc], ones_b, ly_ssum, ly_ssq, spool, f"y{j}", bf16,
                 accum=(j == 0, j == 3))

        # --- after j0: emit deferred z-branch + post weight loads ---------
        if j == 0:
            Wout_b, Wfc1_b, Wfc2_b = [], [], []
            for kk in range(4):
                tb = persist.tile([128, DIM], bf16, tag=f"Wout_b{kk}",
                                  name=f"Wout_b{kk}")
                gdma(tb[:], T["W_out"][kk * 128:(kk + 1) * 128, :])
                Wout_b.append(tb)
            for kk in range(2):
                tb = persist.tile([128, HID], bf16, tag=f"Wfc1_b{kk}",
                                  name=f"Wfc1_b{kk}")
                gdma(tb[:], T["W_fc1"][kk * 128:(kk + 1) * 128, :])
                Wfc1_b.append(tb)
            for kk in range(8):
                tb = persist.tile([128, DIM], bf16, tag=f"Wfc2_b{kk}",
                                  name=f"Wfc2_b{kk}")
                gdma(tb[:], T["W_fc2"][kk * 128:(kk + 1) * 128, :])
                Wfc2_b.append(tb)
            # z branch: LN(row) stats, modulate, z-projection, silu
            ssum_r = statsp[:, 16:24]
            ssq_r = statsp[:, 24:32]
            pe_stats(xTrb, ones_b, ssum_r, ssq_r, spool, "r", bf16)
            mu_r, rstd_r = stats_tail(ssum_r, ssq_r, DIM, "r", spool, stp, bf16)
            hTr = []
            for cc in range(2):
                t1 = spool.tile([128, L], bf16, tag="hscr2", name=f"hs1_r{cc}",
                                bufs=2)
                nc.vector.tensor_tensor(t1[:], xTrb[cc][:], mu_r[:], SUB)
                t2 = spool.tile([128, L], bf16, tag="hscr2", name=f"hs2_r{cc}",
                                bufs=2)
                nc.vector.tensor_tensor(t2[:], t1[:], rstd_r[:], MUL)
                hb = ypool.tile([128, L], bf16, tag=f"hTr{cc}", name=f"hTr{cc}")
                nc.scalar.activation(hb[:], t2[:], AF.Identity,
                                     bias=mcol(0, cc), scale=s1_msa[:, cc:cc + 1])
                hTr.append(hb)
            for jz in range(4):
                ppz = argp.tile([128, L], f32, tag="arg", name=f"ppz{jz}")
                for kk in range(2):
                    for th in range(2):
                        nc.tensor.matmul(ppz[:, th * 512:(th + 1) * 512],
                                         Wz[kk][:, jz * 128:(jz + 1) * 128],
                                         hTr[kk][:, th * 512:(th + 1) * 512],
                                         start=(kk == 0), stop=(kk == 1))
                sz = ypool.tile([128, L], bf16, tag=f"siluz{jz}", name=f"siluz{jz}")
                nc.scalar.activation(sz[:], ppz[:], AF.Silu,
                                     bias=smallt[:, SP_BZ + jz:SP_BZ + jz + 1])
                siluz[jz] = sz

    # ---------------- S12..S16: post phase --------------------------------
    ymu_bc, yrstd_bc = stats_tail(ly_ssum, ly_ssq, DI, "y", spool, stp, bf16)

    gated = []
    for j in range(4):
        t1 = spool.tile([128, L], bf16, tag="psc", name=f"lny1_{j}", bufs=3)
        nc.vector.tensor_tensor(t1[:], y_row[j][:], ymu_bc[:], SUB)
        t2 = spool.tile([128, L], bf16, tag="psc", name=f"lny2_{j}", bufs=3)
        peng = nc.gpsimd if (j % 2 == 0) else nc.vector
        peng.tensor_tensor(t2[:], t1[:], yrstd_bc[:], MUL)
        t3 = spool.tile([128, L], bf16, tag="psc", name=f"lny3_{j}", bufs=3)
        nc.scalar.activation(t3[:], t2[:], AF.Identity,
                             bias=smallt[:, SP_LNB + j:SP_LNB + j + 1],
                             scale=smallt[:, SP_LNW + j:SP_LNW + j + 1])
        gt = ypool.tile([128, L], bf16, tag=f"gated{j}", name=f"gated{j}")
        qeng = nc.gpsimd if (j % 2 == 1) else nc.vector
        qeng.tensor_tensor(gt[:], t3[:], siluz[j][:], MUL)
        gated.append(gt)

    x2T = []
    for cc in range(2):
        php = argp.tile([128, L], f32, tag="arg", name=f"php{cc}")
        for kk in range(4):
            for th in range(2):
                nc.tensor.matmul(php[:, th * 512:(th + 1) * 512],
                                 Wout_b[kk][:, cc * 128:(cc + 1) * 128],
                                 gated[kk][:, th * 512:(th + 1) * 512],
                                 start=(kk == 0), stop=(kk == 3))
        t1 = spool.tile([128, L], f32, tag="pscf", name=f"hyg{cc}", bufs=2)
        nc.scalar.activation(t1[:], php[:], AF.Identity,
                             bias=gb_out[:, cc:cc + 1], scale=mcol(2, cc))
        x2 = ypool.tile([128, L], f32, tag=f"x2T{cc}", name=f"x2T{cc}")
        nc.vector.tensor_tensor(x2[:], t1[:], xTrb[cc][:], ADD)
        x2T.append(x2)

    x2sum = statsp[:, 32:40]
    x2ssq = statsp[:, 40:48]
    pe_stats(x2T, ones_f, x2sum, x2ssq, spool, "x2", f32)
    x2mu_bc, x2rstd_bc = stats_tail(x2sum, x2ssq, DIM, "x2", spool, stp, bf16)

    mT = []
    for cc in range(2):
        t1 = spool.tile([128, L], f32, tag="pscf", name=f"m1_{cc}", bufs=2)
        qeng = nc.gpsimd if (cc % 2 == 0) else nc.vector
        qeng.tensor_tensor(t1[:], x2T[cc][:], x2mu_bc[:], SUB)
        t2 = spool.tile([128, L], f32, tag="pscf", name=f"m2_{cc}", bufs=2)
        qeng.tensor_tensor(t2[:], t1[:], x2rstd_bc[:], MUL)
        mb = ypool.tile([128, L], bf16, tag=f"mT{cc}", name=f"mT{cc}")
        nc.scalar.activation(mb[:], t2[:], AF.Identity,
                             bias=mcol(3, cc), scale=s1_mlp[:, cc:cc + 1])
        mT.append(mb)

    pfc2 = [argp.tile([128, L], f32, tag="arg", name=f"pfc2_{cc}")
            for cc in range(2)]
    for j8 in range(8):
        pfc = yps.tile([128, L], f32, tag="ypt", name=f"pfc1_{j8}")
        for kk in range(2):
            for th in range(2):
                nc.tensor.matmul(pfc[:, th * 512:(th + 1) * 512],
                                 Wfc1_b[kk][:, j8 * 128:(j8 + 1) * 128],
                                 mT[kk][:, th * 512:(th + 1) * 512],
                                 start=(kk == 0), stop=(kk == 1))
        gl = ypool.tile([128, L], bf16, tag="gelu", name=f"gelu{j8}", bufs=2)
        nc.scalar.activation(gl[:], pfc[:], AF.Gelu_apprx_tanh,
                             bias=smallt[:, SP_BFC1 + j8:SP_BFC1 + j8 + 1])
        for cc in range(2):
            for th in range(2):
                nc.tensor.matmul(pfc2[cc][:, th * 512:(th + 1) * 512],
                                 Wfc2_b[j8][:, cc * 128:(cc + 1) * 128],
                                 gl[:, th * 512:(th + 1) * 512],
                                 start=(j8 == 0), stop=(j8 == 7))

    for cc in range(2):
        t1 = spool.tile([128, L], f32, tag="pscf", name=f"mlpg{cc}", bufs=2)
        nc.scalar.activation(t1[:], pfc2[cc][:], AF.Identity,
                             bias=gb_fc2[:, cc:cc + 1], scale=mcol(5, cc))
        o = spool.tile([128, L], f32, tag="outTt", name=f"outT{cc}", bufs=1)
        nc.vector.tensor_tensor(o[:], t1[:], x2T[cc][:], ADD)
        dma(T["outT"][cc * 128:(cc + 1) * 128, :], o[:])

    scan_st.close()
    wstack.close()
    perstack.close()


# ---------------------------------------------------------------------------
# Host side
_PROGRAM = None


def _get_program():
    global _PROGRAM
    if _PROGRAM is None:
        _PROGRAM = build_program()
    return _PROGRAM


def _q_img(x, k):
    img = x.reshape(Hs, Ws, -1)
    if k == 0:
        out = img
    elif k == 1:
        out = img.transpose(1, 0, 2)
    elif k == 2:
        out = img[::-1, ::-1]
    else:
        out = img.transpose(1, 0, 2)[::-1, ::-1]
    return np.ascontiguousarray(out.reshape(L, -1))


def _conv_w_q(w, k):
    if k == 0:
        return w
    if k == 1:
        return np.ascontiguousarray(w.transpose(1, 0, 2))
    if k == 2:
        return np.ascontiguousarray(w[::-1, ::-1])
    return np.ascontiguousarray(w.transpose(1, 0, 2)[::-1, ::-1])


def _col128(v, ncols):
    return np.ascontiguousarray(v.reshape(ncols, 128).T)


def _ind9():
    ind = np.zeros((34, 34), np.float32)
    ind[1:33, 1:33] = 1.0
    ind = ind.reshape(PADW)
    out = np.zeros((9, PADW), np.float32)
    offs = [(dy - 1) * 34 + (dx - 1) for dy in range(3) for dx in range(3)]
    for tap, off in enumerate(offs):
        q = np.arange(PADW) + off
        valid = (q >= 0) & (q < PADW)
        out[tap, valid] = ind[q[valid]]
    return out


def prep_inputs(inputs):
    inp = {k: np.asarray(v, dtype=np.float32) for k, v in inputs.items()}
    x, c = inp["x"], inp["c"]

    shared = {}
    shared["W_ada"] = inp["W_ada"]
    shared["b_ada"] = inp["b_ada"].reshape(1, 6 * DIM)
    W_in = inp["W_in"]
    W_in_xi = np.ascontiguousarray(W_in[:, :DI])
    shared["W_in_z"] = np.ascontiguousarray(W_in[:, DI:])
    shared["ident"] = np.eye(128, dtype=np.float32)
    p = np.arange(128)
    sel2 = np.zeros((2, 128, 128), np.float32)
    for par in range(2):
        sel2[par, p % 64 + par * 64, p] = 1.0
    shared["sel2"] = sel2
    ys = np.zeros((128, 64), np.float32)
    ys[p, p % 64] = 1.0
    shared["ysel"] = ys
    shared["ind9"] = _ind9()
    shared["W_out"] = inp["W_out"]
    shared["W_fc1"] = inp["W_fc1"]
    shared["W_fc2"] = inp["W_fc2"]

    b_in = inp["b_in"]
    in_maps = []
    for core in range(8):
        b, k = core // 4, core % 4
        m = dict(shared)
        xb = x[b]
        xpre = _q_img(xb, k)
        m["xT_pre"] = np.ascontiguousarray(xpre.T)
        m["xT_row"] = np.ascontiguousarray(xb.T)
        m["c_vec"] = c[b].reshape(1, DIM)

        cw = _conv_w_q(inp["conv_w"].reshape(3, 3, DI), k).reshape(9, DI)
        m["W9"] = np.ascontiguousarray(W_in_xi[None, :, :] * cw[:, None, :])
        m["b9"] = np.ascontiguousarray(b_in[None, :DI] * cw)

        Wxp = inp["W_xproj"][k]                           # (DI, 144) cols [dtr,B,C]
        m["W_xp"] = np.ascontiguousarray(
            np.concatenate([Wxp[:, DTR:DTR + DS], Wxp[:, DTR + DS:], Wxp[:, :DTR]],
                           axis=1))
        m["W_dtm"] = np.ascontiguousarray(inp["W_dt"][k])

        sp = np.zeros((128, SP_COLS), np.float32)
        sp[:, SP_BZ:SP_BZ + 4] = _col128(b_in[DI:], 4)
        sp[:, SP_CVB:SP_CVB + 4] = _col128(inp["conv_b"], 4)
        sp[:, SP_DTB:SP_DTB + 4] = _col128(inp["dt_bias"][k], 4)
        sp[:, SP_DP:SP_DP + 4] = _col128(inp["Dp"][k], 4)
        sp[:, SP_LNW:SP_LNW + 4] = _col128(inp["ln_w"], 4)
        sp[:, SP_LNB:SP_LNB + 4] = _col128(inp["ln_b"], 4)
        sp[:, SP_BOUT:SP_BOUT + 2] = _col128(inp["b_out"], 2)
        sp[:, SP_BFC1:SP_BFC1 + 8] = _col128(inp["b_fc1"], 8)
        sp[:, SP_BFC2:SP_BFC2 + 2] = _col128(inp["b_fc2"], 2)
        alog = inp["A_log"][k]                            # (DI, DS)
        acolsv = np.zeros((128, 256), np.float32)
        for g in range(8):
            for i in range(NPAIRS):
                acolsv[:, g * 32 + i] = alog[g * 64 + (p % 64), 2 * i + (p // 64)]
        sp[:, SP_ALOG:SP_ALOG + 256] = acolsv
        m["smallpack"] = sp
        in_maps.append(m)
    return in_maps


def kernel(**inputs):
    nc = _get_program()
    in_maps = prep_inputs(inputs)
    res = run_bass_kernel_spmd(nc, in_maps, list(range(8)))
    out = np.zeros((B, L, DIM), np.float32)
    for b in range(B):
        out[b] = res.results[4 * b]["outT"].T
    return out


# revision 14
# speedup vs baseline: 1.0776x; 1.0776x over previous
"""DiM block (Mamba-style selective-scan transformer block) on 8 TRN2 cores.

Sharding: core i handles (b = i//4, k = i%4) — one batch sample and one of
the 4 scan directions. The spatial permutation q_k is pushed onto
host-prepared inputs so ONE SPMD program serves all 8 cores.

vs the original version:
- Depthwise 3x3 conv is folded into the input projection: host precomputes
  per-tap weights W9[tap] = W_in_xi * conv_w[tap], the kernel matmuls 9
  shifted slices of a zero-padded (guarded) hT tile, accumulating in PSUM.
  The conv bias validity at image borders rides a 9-partition indicator
  matmul (b9 x ind9).
- Scan runs j-major (d-group outer) with y accumulated in PSUM across all
  chunks; each j's ys is AllGathered as soon as it finishes so 3 of the 4
  collectives overlap the remaining scan.
- LN stats are computed with PE ones-matmuls (partition reduction) instead
  of loading extra x copies; the z/silu branch and the post-phase weight
  loads are deferred into the scan window.
- Weight loads use gpsimd casting DMAs (f32 dram -> bf16 SBUF directly).
"""
import json
import sys

sys.path.insert(0, "/opt/trn_rl_repo")

import numpy as np
import concourse.bass as bass
import concourse.mybir as mybir
import concourse.tile as tile
from concourse.bass_utils import run_bass_kernel_spmd

# ---------------------------------------------------------------------------
# Workaround: this walrus build rejects instructions carrying >1 embedded
# sem-wait. Split extra waits onto same-engine NoOps at BIR serialization.
_MAXW = 1
_wsplit_counter = [0]


def _split_multi_waits(bir: dict) -> dict:
    for fn in bir.get("functions", []):
        for bb in fn.get("blocks", []):
            insts = bb.get("instructions", [])
            if not any(
                len((i.get("sync_info") or {}).get("on_wait") or []) > _MAXW
                for i in insts
            ):
                continue
            out = []
            for inst in insts:
                si = inst.get("sync_info")
                waits = (si or {}).get("on_wait") or []
                if len(waits) > _MAXW and inst.get("engine"):
                    for w in waits[:-_MAXW]:
                        _wsplit_counter[0] += 1
                        out.append({
                            "debug": inst.get("debug", 0),
                            "engine": inst["engine"],
                            "ins": [], "outs": [],
                            "name": f"I-wsplit-{_wsplit_counter[0]}",
                            "opcode": "NoOp",
                            "sync_info": {"on_update": [], "on_wait": [w]},
                        })
                    si["on_wait"] = waits[-_MAXW:]
                out.append(inst)
            bb["instructions"] = out
    return bir


_orig_to_json_bytes = bass.Bass.to_json_bytes


def _patched_to_json_bytes(self) -> bytes:
    j = json.loads(_orig_to_json_bytes(self))
    _split_multi_waits(j)
    return json.dumps(j).encode()


bass.Bass.to_json_bytes = _patched_to_json_bytes

# ---------------------------------------------------------------------------
B, Hs, Ws, DIM = 2, 32, 32, 256
L = Hs * Ws
DI = 2 * DIM
DS = 64
DTR = DIM // 16
K = 4
HID = 4 * DIM

f32 = mybir.dt.float32
bf16 = mybir.dt.bfloat16
MUL = mybir.AluOpType.mult
ADD = mybir.AluOpType.add
SUB = mybir.AluOpType.subtract
BYP = mybir.AluOpType.bypass
AF = mybir.ActivationFunctionType
AX = mybir.AxisListType

EPS = 1e-6
NPAIRS = DS // 2          # 32 n-pairs
CHUNK = 4                 # n-pairs per chunk
NCHUNK = NPAIRS // CHUNK  # 8

G = 35                    # guard columns around the padded conv image
PADW = 34 * 34            # 1156
# smallpack column map
SP_BZ, SP_CVB, SP_DTB, SP_DP, SP_LNW, SP_LNB = 0, 4, 8, 12, 16, 20
SP_BOUT, SP_BFC1, SP_BFC2, SP_ALOG = 24, 26, 34, 36
SP_COLS = 36 + 256

XIN_DVE_MOD = 7           # 1-in-7 xin muls run on DVE, rest on Pool


def build_program():
    nc = bass.Bass()

    def din(name, shape, dt=f32):
        return nc.dram_tensor(name, list(shape), dt, kind="ExternalInput")

    T = {}
    T["xT_pre"] = din("xT_pre", (DIM, L))
    T["xT_row"] = din("xT_row", (DIM, L))
    T["c_vec"] = din("c_vec", (1, DIM))
    T["W_ada"] = din("W_ada", (DIM, 6 * DIM))
    T["b_ada"] = din("b_ada", (1, 6 * DIM))
    T["W9"] = din("W9", (9, DIM, DI))         # conv-tap-folded W_in_xi
    T["b9"] = din("b9", (9, DI))              # conv-tap-folded b_in_xi
    T["ind9"] = din("ind9", (9, PADW))        # shifted interior indicators
    T["W_in_z"] = din("W_in_z", (DIM, DI))
    T["W_xp"] = din("W_xp", (DI, 144))        # cols reordered [B(64), C(64), dtr(16)]
    T["W_dtm"] = din("W_dtm", (DTR, DI))
    T["smallpack"] = din("smallpack", (128, SP_COLS))
    T["sel2"] = din("sel2", (2, 128, 128))    # [par][k,p] = (k == p%64 + par*64)
    T["ysel"] = din("ysel", (128, 64))        # [p, d] = (p%64 == d)
    T["ident"] = din("ident", (128, 128))
    T["W_out"] = din("W_out", (DI, DIM))
    T["W_fc1"] = din("W_fc1", (DIM, HID))
    T["W_fc2"] = din("W_fc2", (HID, DIM))

    T["outT"] = nc.dram_tensor("outT", [DIM, L], f32, kind="ExternalOutput")
    for j in range(4):
        T[f"ys_l{j}"] = nc.dram_tensor(f"ys_l{j}", [128, L], bf16)
        T[f"ys_g{j}"] = nc.dram_tensor(f"ys_g{j}", [4, 128, L], bf16)
    T["mod_scr"] = nc.dram_tensor("mod_scr", [1792], f32)

    with tile.TileContext(nc) as tc:
        _build_body(nc, tc, T)
    return nc


def _build_body(nc, tc, T):
    from contextlib import ExitStack

    dma = nc.sync.dma_start

    perstack = ExitStack()
    persist = perstack.enter_context(tc.tile_pool(name="persist", bufs=1))
    wstack = ExitStack()
    wp = wstack.enter_context(tc.tile_pool(name="weights", bufs=1))
    prestack = ExitStack()
    prew = prestack.enter_context(tc.tile_pool(name="prew", bufs=1))
    work = prestack.enter_context(tc.tile_pool(name="work", bufs=1))
    pre_ps = ExitStack()
    psA = pre_ps.enter_context(tc.tile_pool(name="ps_pre", bufs=1, space="PSUM"))

    # ---------------- S0: loads ------------------------------------------
    c_t = work.tile([1, DIM], f32, tag="c_t", name="c_t")
    dma(c_t[:], T["c_vec"][:, :])
    smallt = persist.tile([128, SP_COLS], f32, tag="smallt", name="smallt")
    dma(smallt[:], T["smallpack"][:, :])
    def load_cast(dram_slice, rows, cols, pool, tag, name, eng=None, sbufs=3):
        st = prew.tile([rows, cols], f32, tag="stage", name=f"st_{name}",
                       bufs=sbufs)
        dma(st[:], dram_slice)
        tb = pool.tile([rows, cols], bf16, tag=tag, name=name)
        if eng is None:
            nc.scalar.copy(tb[:], st[:])
        else:
            eng.tensor_copy(tb[:], st[:])
        return tb

    Wada = [load_cast(T["W_ada"][j * 128:(j + 1) * 128, :], 128, 6 * DIM,
                      prew, f"Wada_b{j}", f"Wada_b{j}") for j in range(2)]
    bada = work.tile([1, 6 * DIM], f32, tag="bada", name="bada")
    dma(bada[:], T["b_ada"][:, :])
    xTp = [load_cast(T["xT_pre"][cc * 128:(cc + 1) * 128, :], 128, L,
                     prew, f"xTp{cc}", f"xTp{cc}", eng=nc.vector)
           for cc in range(2)]
    identf = persist.tile([128, 128], f32, tag="identf", name="identf")
    dma(identf[:], T["ident"][:, :])
    W9b = [load_cast(T["W9"][tap, kk * 128:(kk + 1) * 128, :], 128, DI,
                     prew, f"W9_{tap}_{kk}", f"W9_{tap}_{kk}")
           for tap in range(9) for kk in range(2)]
    b9b = load_cast(T["b9"][:, :], 9, DI, prew, "b9b", "b9b")
    ind9b = load_cast(T["ind9"][:, :], 9, PADW, prew, "ind9b", "ind9b")
    sel_b = [load_cast(T["sel2"][par, :, :], 128, 128, persist,
                       f"sel_b{par}", f"sel_b{par}") for par in range(2)]
    ysel_b = load_cast(T["ysel"][:, :], 128, 64, persist, "ysel_b", "ysel_b")
    Wxp = [load_cast(T["W_xp"][kk * 128:(kk + 1) * 128, :], 128, 144, persist,
                     f"Wxp_b{kk}", f"Wxp_b{kk}") for kk in range(4)]
    Wdt = load_cast(T["W_dtm"][:, :], DTR, DI, persist, "Wdt_b", "Wdt_b")
    Wz = [load_cast(T["W_in_z"][kk * 128:(kk + 1) * 128, :], 128, DI, persist,
                    f"Wz_b{kk}", f"Wz_b{kk}") for kk in range(2)]
    xTrb = [load_cast(T["xT_row"][cc * 128:(cc + 1) * 128, :], 128, L, persist,
                      f"xTrb{cc}", f"xTrb{cc}", eng=nc.vector)
            for cc in range(2)]

    eps_col = persist.tile([128, 1], f32, tag="eps_col", name="eps_col")
    nc.gpsimd.memset(eps_col[:], EPS)
    ones_b = persist.tile([128, 1], bf16, tag="ones_b", name="ones_b")
    nc.gpsimd.memset(ones_b[:], 1.0)
    ones_f = persist.tile([128, 1], f32, tag="ones_f", name="ones_f")
    nc.gpsimd.memset(ones_f[:], 1.0)
    acols = persist.tile([128, 256], f32, tag="acols", name="acols")
    nc.scalar.activation(acols[:], smallt[:, SP_ALOG:SP_ALOG + 256], AF.Exp)

    # ---------------- S1: adaLN modulation vector -------------------------
    c_silu = work.tile([1, DIM], f32, tag="c_silu", name="c_silu")
    nc.scalar.activation(c_silu[:], c_t[:], AF.Silu)
    c_col = work.tile([128, 2], f32, tag="c_col", name="c_col")
    dma(T["mod_scr"][1536:1792], c_silu[0:1, :])
    dma(c_col[:], T["mod_scr"][1536:1792].rearrange("(j p) -> p j", j=2, p=128))
    c_colb = work.tile([128, 2], bf16, tag="c_colb", name="c_colb")
    nc.vector.tensor_copy(c_colb[:], c_col[:])

    mod = work.tile([1, 6 * DIM], f32, tag="mod", name="mod")
    for fb in range(3):
        pmod = psA.tile([1, 512], f32, tag="pmod", name=f"pmod{fb}", bufs=1)
        for kk in range(2):
            nc.tensor.matmul(pmod[:], c_colb[:, kk:kk + 1],
                             Wada[kk][:, fb * 512:(fb + 1) * 512],
                             start=(kk == 0), stop=(kk == 1))
        nc.vector.tensor_tensor(mod[:, fb * 512:(fb + 1) * 512], pmod[:],
                                bada[:, fb * 512:(fb + 1) * 512], ADD)
    dma(T["mod_scr"][0:1536], mod[0:1, :])
    mcolt = persist.tile([128, 12], f32, tag="mcolt", name="mcolt")
    dma(mcolt[:], T["mod_scr"][0:1536].rearrange("(a p) -> p a", a=12, p=128))

    def mcol(i6, cc):
        return mcolt[:, i6 * 2 + cc:i6 * 2 + cc + 1]

    s1_msa = persist.tile([128, 2], f32, tag="s1_msa", name="s1_msa")
    nc.scalar.activation(s1_msa[:], mcolt[:, 2:4], AF.Identity, bias=1.0)
    s1_mlp = persist.tile([128, 2], f32, tag="s1_mlp", name="s1_mlp")
    nc.scalar.activation(s1_mlp[:], mcolt[:, 8:10], AF.Identity, bias=1.0)
    gb_out = persist.tile([128, 2], f32, tag="gb_out", name="gb_out")
    nc.vector.tensor_tensor(gb_out[:], mcolt[:, 4:6],
                            smallt[:, SP_BOUT:SP_BOUT + 2], MUL)
    gb_fc2 = persist.tile([128, 2], f32, tag="gb_fc2", name="gb_fc2")
    nc.vector.tensor_tensor(gb_fc2[:], mcolt[:, 10:12],
                            smallt[:, SP_BFC2:SP_BFC2 + 2], MUL)

    # ---------------- shared LN helpers (stats via PE) --------------------
    def pe_stats(tiles, onecol, ssum, ssq, pool, name, sq_dt, accum=None):
        """Per-token sum and sum-of-squares over partitions of `tiles`.
        tiles: list of (128, L) tiles whose partitions are feature rows.
        Writes into PSUM tiles ssum/ssq (128, 8). accum: (start, stop) flags
        override for cross-call accumulation."""
        n = len(tiles)
        sqt = []
        for i, t in enumerate(tiles):
            sq = pool.tile([128, L], sq_dt, tag=f"sqt{sq_dt}", name=f"sq_{name}{i}",
                           bufs=2)
            nc.scalar.activation(sq[:], t[:], AF.Square)
            sqt.append(sq)
        st0, st1 = (True, True) if accum is None else accum
        for tb in range(8):
            for i in range(n):
                nc.tensor.matmul(ssum[:, tb:tb + 1],
                                 tiles[i][:, tb * 128:(tb + 1) * 128], onecol[:],
                                 start=(st0 and i == 0), stop=(st1 and i == n - 1))
                nc.tensor.matmul(ssq[:, tb:tb + 1],
                                 sqt[i][:, tb * 128:(tb + 1) * 128], onecol[:],
                                 start=(st0 and i == 0), stop=(st1 and i == n - 1))

    def bcast_cols(stat, name, pool, psum_pool, tag, dt):
        """(128,8) per-token stat -> (128,L) all-partition broadcast tile."""
        statT_p = psum_pool.tile([8, 128], f32, tag="statT_p", name=f"sTp_{name}",
                                 bufs=1)
        nc.tensor.transpose(statT_p[:], stat[:], identf[:])
        statT = pool.tile([8, 128], dt, tag=f"statT{dt}", name=f"sT_{name}", bufs=1)
        nc.scalar.copy(statT[:], statT_p[:])
        row2 = pool.tile([2, L], dt, tag=f"row2{dt}", name=f"r2_{name}", bufs=1)
        dma(row2[0:1, :], statT[:, :])
        dma(row2[1:2, :], statT[:, :])
        bc = pool.tile([128, L], dt, tag=f"{tag}{dt}", name=f"bc_{name}", bufs=1)
        dma(bc[:], row2[:, :].partition_broadcast(64).rearrange("n d f -> d n f"))
        return bc

    def stats_tail(ssum, ssq, dim, name, pool, psum_pool, dt):
        mu = pool.tile([128, 8], f32, tag="pmu", name=f"pmu_{name}", bufs=2)
        nc.vector.tensor_scalar_mul(mu[:], ssum[:], 1.0 / dim)
        mu2 = pool.tile([128, 8], f32, tag="pmu2", name=f"pmu2_{name}", bufs=2)
        nc.vector.tensor_tensor(mu2[:], mu[:], mu[:], MUL)
        var = pool.tile([128, 8], f32, tag="pvar", name=f"pvar_{name}", bufs=2)
        nc.vector.scalar_tensor_tensor(var[:], ssq[:], 1.0 / dim, mu2[:], MUL, SUB)
        std = pool.tile([128, 8], f32, tag="pstd", name=f"pstd_{name}", bufs=2)
        nc.scalar.activation(std[:], var[:], AF.Sqrt, bias=eps_col[:, 0:1])
        rstd = pool.tile([128, 8], f32, tag="prstd", name=f"prstd_{name}", bufs=2)
        nc.vector.reciprocal(rstd[:], std[:])
        mu_bc = bcast_cols(mu, f"{name}m", pool, psum_pool, "bcA", dt)
        rstd_bc = bcast_cols(rstd, f"{name}r", pool, psum_pool, "bcB", dt)
        return mu_bc, rstd_bc

    # ---------------- S2: LN1(pre) + modulate into padded tiles ----------
    ssum_p = psA.tile([128, 8], f32, tag="ssum_p", name="ssum_p")
    ssq_p = psA.tile([128, 8], f32, tag="ssq_p", name="ssq_p")
    pe_stats(xTp, ones_b, ssum_p, ssq_p, work, "p", bf16)
    mu_p, rstd_p = stats_tail(ssum_p, ssq_p, DIM, "p", work, psA, bf16)

    hpad = []
    for cc in range(2):
        hp = prew.tile([128, G + PADW + G], bf16, tag=f"hpad{cc}", name=f"hpad{cc}")
        nc.gpsimd.memset(hp[:], 0.0)
        t1 = work.tile([128, L], bf16, tag="hscr", name=f"hs1_p{cc}", bufs=2)
        nc.vector.tensor_tensor(t1[:], xTp[cc][:], mu_p[:], SUB)
        t2 = work.tile([128, L], bf16, tag="hscr", name=f"hs2_p{cc}", bufs=2)
        nc.vector.tensor_tensor(t2[:], t1[:], rstd_p[:], MUL)
        interior = (hp[:, G:G + PADW]
                    .rearrange("p (H W) -> p H W", H=34, W=34)[:, 1:33, 1:33])
        nc.scalar.activation(interior, t2[:].rearrange("p (h w) -> p h w", h=32, w=32),
                             AF.Identity, bias=mcol(0, cc), scale=s1_msa[:, cc:cc + 1])
        hpad.append(hp)

    pre_ps.close()

    # ---------------- S4: conv-proj on PE + B/C/dtr projections -----------
    conv_ps = ExitStack()
    cpp = conv_ps.enter_context(tc.tile_pool(name="ps_conv", bufs=1, space="PSUM"))
    bcpp = conv_ps.enter_context(tc.tile_pool(name="ps_bc", bufs=1, space="PSUM"))
    CCHUNKS = [(0, 512), (512, 1024), (1024, PADW)]
    OFFS = [(dy - 1) * 34 + (dx - 1) for dy in range(3) for dx in range(3)]

    ppbc = bcpp.tile([128, L], f32, tag="ppbc", name="ppbc")
    ppdtr = bcpp.tile([DTR, L], f32, tag="ppdtr", name="ppdtr")
    xsT = []
    for j in range(4):
        ppcv = cpp.tile([128, PADW], f32, tag="ppcv", name=f"ppcv{j}", bufs=1)
        for (c0, c1) in CCHUNKS:
            nmm = 0
            for tap in range(9):
                for kk in range(2):
                    nc.tensor.matmul(
                        ppcv[:, c0:c1],
                        W9b[tap * 2 + kk][:, j * 128:(j + 1) * 128],
                        hpad[kk][:, G + c0 + OFFS[tap]:G + c1 + OFFS[tap]],
                        start=(nmm == 0), stop=False)
                    nmm += 1
            nc.tensor.matmul(ppcv[:, c0:c1], b9b[:, j * 128:(j + 1) * 128],
                             ind9b[:, c0:c1], start=False, stop=True)
        xs = wp.tile([128, L], bf16, tag=f"xsT{j}", name=f"xsT{j}")
        inter = (ppcv[:, :].rearrange("p (H W) -> p H W", H=34, W=34)
                 [:, 1:33, 1:33])
        nc.scalar.activation(xs[:].rearrange("p (h w) -> p h w", h=32, w=32),
                             inter, AF.Silu,
                             bias=smallt[:, SP_CVB + j:SP_CVB + j + 1])
        xsT.append(xs)
        # B/C and dt_r projections accumulate over j
        for th in range(2):
            nc.tensor.matmul(ppbc[:, th * 512:(th + 1) * 512],
                             Wxp[j][:, 0:128],
                             xs[:, th * 512:(th + 1) * 512],
                             start=(j == 0), stop=(j == 3))
            nc.tensor.matmul(ppdtr[:, th * 512:(th + 1) * 512],
                             Wxp[j][:, 128:144],
                             xs[:, th * 512:(th + 1) * 512],
                             start=(j == 0), stop=(j == 3))

    bc_t = wp.tile([128, L], bf16, tag="bc_t", name="bc_t")
    nc.scalar.copy(bc_t[:], ppbc[:])
    dtr_t = wp.tile([DTR, L], bf16, tag="dtr_t", name="dtr_t")
    nc.scalar.copy(dtr_t[:], ppdtr[:])
    conv_ps.close()
    prestack.close()

    # ---------------- S7/S8: scan stage, j-major --------------------------
    scan_st = ExitStack()
    argp = scan_st.enter_context(tc.tile_pool(name="argp", bufs=2, space="PSUM"))
    yps = scan_st.enter_context(tc.tile_pool(name="yps", bufs=1, space="PSUM"))
    stp = scan_st.enter_context(tc.tile_pool(name="stp", bufs=1, space="PSUM"))
    spool = scan_st.enter_context(tc.tile_pool(name="spool", bufs=2))
    bpool = scan_st.enter_context(tc.tile_pool(name="bpool", bufs=1))
    ypool = scan_st.enter_context(tc.tile_pool(name="ypool", bufs=1))

    statsp = stp.tile([128, 48], f32, tag="statsp", name="statsp")
    ly_ssum = statsp[:, 0:8]
    ly_ssq = statsp[:, 8:16]

    y_row = [None] * 4
    siluz = [None] * 4

    def emit_combine(j):
        ysk = []
        for k in range(4):
            t16 = ypool.tile([128, L], bf16, tag="ysk", name=f"ysk{j}_{k}", bufs=3)
            dma(t16[:], T[f"ys_g{j}"][k, :, :])
            ysk.append(t16)
        acc = ypool.tile([128, L], bf16, tag="yrow_s", name=f"yrow{j}_0", bufs=2)
        nc.vector.tensor_copy(acc[:], ysk[0][:])
        for k in (1, 2, 3):
            nacc = ypool.tile([128, L], bf16,
                              tag=f"yrowf{j}" if k == 3 else "yrow_s",
                              name=f"yrow{j}_{k}", bufs=1 if k == 3 else 2)
            if k == 2:
                nc.vector.tensor_tensor(nacc[:], acc[:], ysk[2][:, ::-1], ADD)
            else:
                srct = ysk[k][:, :] if k == 1 else ysk[k][:, ::-1]
                view = (srct.rearrange("p (w h) -> p w h", w=32, h=32)
                        .rearrange("p w h -> p h w"))
                nc.vector.tensor_tensor(
                    nacc[:].rearrange("p (h w) -> p h w", h=32, w=32),
                    acc[:].rearrange("p (h w) -> p h w", h=32, w=32),
                    view, ADD)
            acc = nacc
        y_row[j] = acc
        pe_stats([acc], ones_b, ly_ssum, ly_ssq, spool, f"y{j}", bf16,
                 accum=(j == 0, j == 3))

    it = 0
    for j in range(4):
        # --- dt chain for this j
        ppd = argp.tile([128, L], f32, tag="arg", name=f"ppdt{j}")
        for th in range(2):
            nc.tensor.matmul(ppd[:, th * 512:(th + 1) * 512],
                             Wdt[:, j * 128:(j + 1) * 128],
                             dtr_t[:, th * 512:(th + 1) * 512],
                             start=True, stop=True)
        spx = spool.tile([128, L], f32, tag="spx", name=f"spx{j}", bufs=1)
        nc.scalar.activation(spx[:], ppd[:], AF.Exp,
                             bias=smallt[:, SP_DTB + j:SP_DTB + j + 1])
        dt_b = spool.tile([128, L], bf16, tag="dtT", name=f"dtT{j}", bufs=2)
        nc.scalar.activation(dt_b[:], spx[:], AF.Ln, bias=1.0)
        ndt = ypool.tile([128, L], bf16, tag="ndt", name=f"ndtT{j}", bufs=2)
        nc.vector.tensor_scalar_mul(ndt[:], dt_b[:], -1.0)
        w_b = spool.tile([128, L], bf16, tag="wTtmp", name=f"wT{j}", bufs=2)
        nc.vector.tensor_tensor(w_b[:], dt_b[:], xsT[j][:], MUL)
        wbc = {}
        for par in range(2):
            g = 2 * j + par
            wsrc = w_b[par * 64:par * 64 + 64, :]
            wb = ypool.tile([128, L], bf16, tag=f"wbc{par}", name=f"wbc{g}", bufs=1)
            dma(wb[0:64, :], wsrc)
            dma(wb[64:128, :], wsrc)
            wbc[par] = wb

        ypt = yps.tile([128, L], f32, tag="ypt", name=f"ypt{j}")
        for ch in range(NCHUNK):
            Bb, Cb = [], []
            for il in range(CHUNK):
                i = ch * CHUNK + il
                bbt = bpool.tile([128, L], bf16, tag="Bb", name=f"Bb{j}_{ch}_{il}",
                                 bufs=5)
                dma(bbt[:], bc_t[2 * i:2 * i + 2, :]
                    .partition_broadcast(64).rearrange("d n f -> n d f"))
                Bb.append(bbt)
                cbt = bpool.tile([128, L], bf16, tag="Cb", name=f"Cb{j}_{ch}_{il}",
                                 bufs=5)
                dma(cbt[:], bc_t[64 + 2 * i:64 + 2 * i + 2, :]
                    .partition_broadcast(64).rearrange("d n f -> n d f"))
                Cb.append(cbt)
            for par in range(2):
                g = 2 * j + par
                for il in range(CHUNK):
                    i = ch * CHUNK + il
                    arg = argp.tile([128, L], f32, tag="arg", name=f"arg{it}")
                    for th in range(2):
                        nc.tensor.matmul(arg[:, th * 512:(th + 1) * 512],
                                         sel_b[par][:],
                                         ndt[:, th * 512:(th + 1) * 512],
                                         start=True, stop=True)
                    dA = spool.tile([128, L], bf16, tag="dA", name=f"dA{it}", bufs=3)
                    nc.scalar.activation(dA[:], arg[:], AF.Exp,
                                         scale=acols[:, g * 32 + i:g * 32 + i + 1])
                    xin = spool.tile([128, L], bf16, tag="xin", name=f"xin{it}",
                                     bufs=4)
                    xeng = nc.vector if (it % XIN_DVE_MOD == 3) else nc.gpsimd
                    xeng.tensor_tensor(xin[:], wbc[par][:], Bb[il][:], MUL)
                    h = spool.tile([128, L], bf16, tag="h", name=f"h{it}", bufs=3)
                    nc.vector.tensor_tensor_scan(h[:], dA[:], xin[:], 0.0, MUL, ADD)
                    yc = spool.tile([128, L], bf16, tag="yc", name=f"yc{it}", bufs=3)
                    nc.vector.tensor_tensor(yc[:], h[:], Cb[il][:], MUL)
                    for th in range(2):
                        nc.tensor.matmul(
                            ypt[par * 64:par * 64 + 64, th * 512:(th + 1) * 512],
                            ysel_b[:], yc[:, th * 512:(th + 1) * 512],
                            start=(ch == 0 and il == 0),
                            stop=(ch == NCHUNK - 1 and il == CHUNK - 1))
                    it += 1

        # --- ys for this j -> DRAM -> AllGather
        ys16 = spool.tile([128, L], bf16, tag="ys16", name=f"ys16_{j}", bufs=2)
        nc.vector.scalar_tensor_tensor(ys16[:], xsT[j][:],
                                       smallt[:, SP_DP + j:SP_DP + j + 1],
                                       ypt[:], MUL, ADD)
        dma(T[f"ys_l{j}"][:, :], ys16[:])
        nc.gpsimd.collective_compute(
            "AllGather", BYP,
            replica_groups=[[0, 1, 2, 3], [4, 5, 6, 7]],
            ins=[T[f"ys_l{j}"][:, :]],
            outs=[T[f"ys_g{j}"][:, :, :]],
        )

        # --- combine for j-1 (its gather landed during this j's scan) -----
        if j >= 1:
            emit_combine(j - 1)

        # --- after j0: emit deferred z-branch + post weight loads ---------
        if j == 0:
            def load_cast2(dram_slice, cols, tag, name):
                st = spool.tile([128, cols], f32, tag="stage2", name=f"s2_{name}",
                               bufs=2)
                dma(st[:], dram_slice)
                tb = persist.tile([128, cols], bf16, tag=tag, name=name)
                nc.scalar.copy(tb[:], st[:])
                return tb

            Wout_b = [load_cast2(T["W_out"][kk * 128:(kk + 1) * 128, :], DIM,
                                 f"Wout_b{kk}", f"Wout_b{kk}") for kk in range(4)]
            Wfc1_b = [load_cast2(T["W_fc1"][kk * 128:(kk + 1) * 128, :], HID,
                                 f"Wfc1_b{kk}", f"Wfc1_b{kk}") for kk in range(2)]
            Wfc2_b = [load_cast2(T["W_fc2"][kk * 128:(kk + 1) * 128, :], DIM,
                                 f"Wfc2_b{kk}", f"Wfc2_b{kk}") for kk in range(8)]
            # z branch: LN(row) stats, modulate, z-projection, silu
            ssum_r = statsp[:, 16:24]
            ssq_r = statsp[:, 24:32]
            pe_stats(xTrb, ones_b, ssum_r, ssq_r, spool, "r", bf16)
            mu_r, rstd_r = stats_tail(ssum_r, ssq_r, DIM, "r", spool, stp, bf16)
            hTr = []
            for cc in range(2):
                t1 = spool.tile([128, L], bf16, tag="hscr2", name=f"hs1_r{cc}",
                                bufs=2)
                nc.vector.tensor_tensor(t1[:], xTrb[cc][:], mu_r[:], SUB)
                t2 = spool.tile([128, L], bf16, tag="hscr2", name=f"hs2_r{cc}",
                                bufs=2)
                nc.vector.tensor_tensor(t2[:], t1[:], rstd_r[:], MUL)
                hb = ypool.tile([128, L], bf16, tag=f"hTr{cc}", name=f"hTr{cc}")
                nc.scalar.activation(hb[:], t2[:], AF.Identity,
                                     bias=mcol(0, cc), scale=s1_msa[:, cc:cc + 1])
                hTr.append(hb)
            for jz in range(4):
                ppz = argp.tile([128, L], f32, tag="arg", name=f"ppz{jz}")
                for kk in range(2):
                    for th in range(2):
                        nc.tensor.matmul(ppz[:, th * 512:(th + 1) * 512],
                                         Wz[kk][:, jz * 128:(jz + 1) * 128],
                                         hTr[kk][:, th * 512:(th + 1) * 512],
                                         start=(kk == 0), stop=(kk == 1))
                sz = ypool.tile([128, L], bf16, tag=f"siluz{jz}", name=f"siluz{jz}")
                nc.scalar.activation(sz[:], ppz[:], AF.Silu,
                                     bias=smallt[:, SP_BZ + jz:SP_BZ + jz + 1])
                siluz[jz] = sz

    emit_combine(3)

    # ---------------- S12..S16: post phase --------------------------------
    ymu_bc, yrstd_bc = stats_tail(ly_ssum, ly_ssq, DI, "y", spool, stp, bf16)

    gated = []
    for j in range(4):
        t1 = spool.tile([128, L], bf16, tag="psc", name=f"lny1_{j}", bufs=3)
        nc.vector.tensor_tensor(t1[:], y_row[j][:], ymu_bc[:], SUB)
        t2 = spool.tile([128, L], bf16, tag="psc", name=f"lny2_{j}", bufs=3)
        peng = nc.gpsimd if (j % 2 == 0) else nc.vector
        peng.tensor_tensor(t2[:], t1[:], yrstd_bc[:], MUL)
        t3 = spool.tile([128, L], bf16, tag="psc", name=f"lny3_{j}", bufs=3)
        nc.scalar.activation(t3[:], t2[:], AF.Identity,
                             bias=smallt[:, SP_LNB + j:SP_LNB + j + 1],
                             scale=smallt[:, SP_LNW + j:SP_LNW + j + 1])
        gt = ypool.tile([128, L], bf16, tag=f"gated{j}", name=f"gated{j}")
        qeng = nc.gpsimd if (j % 2 == 1) else nc.vector
        qeng.tensor_tensor(gt[:], t3[:], siluz[j][:], MUL)
        gated.append(gt)

    x2T = []
    for cc in range(2):
        php = argp.tile([128, L], f32, tag="arg", name=f"php{cc}")
        for kk in range(4):
            for th in range(2):
                nc.tensor.matmul(php[:, th * 512:(th + 1) * 512],
                                 Wout_b[kk][:, cc * 128:(cc + 1) * 128],
                                 gated[kk][:, th * 512:(th + 1) * 512],
                                 start=(kk == 0), stop=(kk == 3))
        t1 = spool.tile([128, L], f32, tag="pscf", name=f"hyg{cc}", bufs=2)
        nc.scalar.activation(t1[:], php[:], AF.Identity,
                             bias=gb_out[:, cc:cc + 1], scale=mcol(2, cc))
        x2 = ypool.tile([128, L], f32, tag=f"x2T{cc}", name=f"x2T{cc}")
        nc.vector.tensor_tensor(x2[:], t1[:], xTrb[cc][:], ADD)
        x2T.append(x2)

    x2sum = statsp[:, 32:40]
    x2ssq = statsp[:, 40:48]
    pe_stats(x2T, ones_f, x2sum, x2ssq, spool, "x2", f32)
    x2mu_bc, x2rstd_bc = stats_tail(x2sum, x2ssq, DIM, "x2", spool, stp, bf16)

    mT = []
    for cc in range(2):
        t1 = spool.tile([128, L], f32, tag="pscf", name=f"m1_{cc}", bufs=2)
        qeng = nc.gpsimd if (cc % 2 == 0) else nc.vector
        qeng.tensor_tensor(t1[:], x2T[cc][:], x2mu_bc[:], SUB)
        t2 = spool.tile([128, L], f32, tag="pscf", name=f"m2_{cc}", bufs=2)
        qeng.tensor_tensor(t2[:], t1[:], x2rstd_bc[:], MUL)
        mb = ypool.tile([128, L], bf16, tag=f"mT{cc}", name=f"mT{cc}")
        nc.scalar.activation(mb[:], t2[:], AF.Identity,
                             bias=mcol(3, cc), scale=s1_mlp[:, cc:cc + 1])
        mT.append(mb)

    pfc2 = [argp.tile([128, L], f32, tag="arg", name=f"pfc2_{cc}")
            for cc in range(2)]
    for j8 in range(8):
        pfc = yps.tile([128, L], f32, tag="ypt", name=f"pfc1_{j8}")
        for kk in range(2):
            for th in range(2):
                nc.tensor.matmul(pfc[:, th * 512:(th + 1) * 512],
                                 Wfc1_b[kk][:, j8 * 128:(j8 + 1) * 128],
                                 mT[kk][:, th * 512:(th + 1) * 512],
                                 start=(kk == 0), stop=(kk == 1))
        gl = ypool.tile([128, L], bf16, tag="gelu", name=f"gelu{j8}", bufs=2)
        nc.scalar.activation(gl[:], pfc[:], AF.Gelu_apprx_tanh,
                             bias=smallt[:, SP_BFC1 + j8:SP_BFC1 + j8 + 1])
        for cc in range(2):
            for th in range(2):
                nc.tensor.matmul(pfc2[cc][:, th * 512:(th + 1) * 512],
                                 Wfc2_b[j8][:, cc * 128:(cc + 1) * 128],
                                 gl[:, th * 512:(th + 1) * 512],
                                 start=(j8 == 0), stop=(j8 == 7))

    for cc in range(2):
        t1 = spool.tile([128, L], f32, tag="pscf", name=f"mlpg{cc}", bufs=2)
        nc.scalar.activation(t1[:], pfc2[cc][:], AF.Identity,
                             bias=gb_fc2[:, cc:cc + 1], scale=mcol(5, cc))
        o = spool.tile([128, L], f32, tag="outTt", name=f"outT{cc}", bufs=1)
        nc.vector.tensor_tensor(o[:], t1[:], x2T[cc][:], ADD)
        dma(T["outT"][cc * 128:(cc + 1) * 128, :], o[:])

    scan_st.close()
    wstack.close()
    perstack.close()


# ---------------------------------------------------------------------------
# Host side
_PROGRAM = None


def _get_program():
    global _PROGRAM
    if _PROGRAM is None:
        _PROGRAM = build_program()
    return _PROGRAM


def _q_img(x, k):
    img = x.reshape(Hs, Ws, -1)
    if k == 0:
        out = img
    elif k == 1:
        out = img.transpose(1, 0, 2)
    elif k == 2:
        out = img[::-1, ::-1]
    else:
        out = img.transpose(1, 0, 2)[::-1, ::-1]
    return np.ascontiguousarray(out.reshape(L, -1))


def _conv_w_q(w, k):
    if k == 0:
        return w
    if k == 1:
        return np.ascontiguousarray(w.transpose(1, 0, 2))
    if k == 2:
        return np.ascontiguousarray(w[::-1, ::-1])
    return np.ascontiguousarray(w.transpose(1, 0, 2)[::-1, ::-1])


def _col128(v, ncols):
    return np.ascontiguousarray(v.reshape(ncols, 128).T)


def _ind9():
    ind = np.zeros((34, 34), np.float32)
    ind[1:33, 1:33] = 1.0
    ind = ind.reshape(PADW)
    out = np.zeros((9, PADW), np.float32)
    offs = [(dy - 1) * 34 + (dx - 1) for dy in range(3) for dx in range(3)]
    for tap, off in enumerate(offs):
        q = np.arange(PADW) + off
        valid = (q >= 0) & (q < PADW)
        out[tap, valid] = ind[q[valid]]
    return out


def prep_inputs(inputs):
    inp = {k: np.asarray(v, dtype=np.float32) for k, v in inputs.items()}
    x, c = inp["x"], inp["c"]

    shared = {}
    shared["W_ada"] = inp["W_ada"]
    shared["b_ada"] = inp["b_ada"].reshape(1, 6 * DIM)
    W_in = inp["W_in"]
    W_in_xi = np.ascontiguousarray(W_in[:, :DI])
    shared["W_in_z"] = np.ascontiguousarray(W_in[:, DI:])
    shared["ident"] = np.eye(128, dtype=np.float32)
    p = np.arange(128)
    sel2 = np.zeros((2, 128, 128), np.float32)
    for par in range(2):
        sel2[par, p % 64 + par * 64, p] = 1.0
    shared["sel2"] = sel2
    ys = np.zeros((128, 64), np.float32)
    ys[p, p % 64] = 1.0
    shared["ysel"] = ys
    shared["ind9"] = _ind9()
    shared["W_out"] = inp["W_out"]
    shared["W_fc1"] = inp["W_fc1"]
    shared["W_fc2"] = inp["W_fc2"]

    b_in = inp["b_in"]
    in_maps = []
    for core in range(8):
        b, k = core // 4, core % 4
        m = dict(shared)
        xb = x[b]
        xpre = _q_img(xb, k)
        m["xT_pre"] = np.ascontiguousarray(xpre.T)
        m["xT_row"] = np.ascontiguousarray(xb.T)
        m["c_vec"] = c[b].reshape(1, DIM)

        cw = _conv_w_q(inp["conv_w"].reshape(3, 3, DI), k).reshape(9, DI)
        m["W9"] = np.ascontiguousarray(W_in_xi[None, :, :] * cw[:, None, :])
        m["b9"] = np.ascontiguousarray(b_in[None, :DI] * cw)

        Wxp = inp["W_xproj"][k]                           # (DI, 144) cols [dtr,B,C]
        m["W_xp"] = np.ascontiguousarray(
            np.concatenate([Wxp[:, DTR:DTR + DS], Wxp[:, DTR + DS:], Wxp[:, :DTR]],
                           axis=1))
        m["W_dtm"] = np.ascontiguousarray(inp["W_dt"][k])

        sp = np.zeros((128, SP_COLS), np.float32)
        sp[:, SP_BZ:SP_BZ + 4] = _col128(b_in[DI:], 4)
        sp[:, SP_CVB:SP_CVB + 4] = _col128(inp["conv_b"], 4)
        sp[:, SP_DTB:SP_DTB + 4] = _col128(inp["dt_bias"][k], 4)
        sp[:, SP_DP:SP_DP + 4] = _col128(inp["Dp"][k], 4)
        sp[:, SP_LNW:SP_LNW + 4] = _col128(inp["ln_w"], 4)
        sp[:, SP_LNB:SP_LNB + 4] = _col128(inp["ln_b"], 4)
        sp[:, SP_BOUT:SP_BOUT + 2] = _col128(inp["b_out"], 2)
        sp[:, SP_BFC1:SP_BFC1 + 8] = _col128(inp["b_fc1"], 8)
        sp[:, SP_BFC2:SP_BFC2 + 2] = _col128(inp["b_fc2"], 2)
        alog = inp["A_log"][k]                            # (DI, DS)
        acolsv = np.zeros((128, 256), np.float32)
        for g in range(8):
            for i in range(NPAIRS):
                acolsv[:, g * 32 + i] = alog[g * 64 + (p % 64), 2 * i + (p // 64)]
        sp[:, SP_ALOG:SP_ALOG + 256] = acolsv
        m["smallpack"] = sp
        in_maps.append(m)
    return in_maps


def kernel(**inputs):
    nc = _get_program()
    in_maps = prep_inputs(inputs)
    res = run_bass_kernel_spmd(nc, in_maps, list(range(8)))
    out = np.zeros((B, L, DIM), np.float32)
    for b in range(B):
        out[b] = res.results[4 * b]["outT"].T
    return out
